# revision 17
# baseline (speedup 1.0000x reference)
"""Trainium2 Bass kernel: batched complex-waveform similarity.

Math: reference computes
    bank = ifft_ortho(freq)                # [T, L] complex
    score = rx @ conj(bank).T              # [B, T] complex
    sim   = (score.re^2 + score.im^2) / temperature

Since the ortho DFT is unitary,  score = fft_ortho(rx) @ conj(freq).T.
So the kernel never builds the bank: it DFTs rx via a 128x128 bf16
matmul, then runs one big complex GEMM [B,L]x[L,T] in bf16 with fp32
PSUM accumulation, and a fused squared-magnitude epilogue in bf16.

Sharding: data-parallel over the rx batch dim across 8 NeuronCores;
freq (as a packed transposed bf16 [L, 2T]) is replicated on every core.

Hard-won schedule notes (from perfetto traces of 5 prior variants):
  * Every dma_start costs its issuing sequencer ~0.6us of serialized
    descriptor generation, and the transfer starts only after that.
    So inputs are PACKED host-side into a handful of wide DMAs, and
    output tiles are paired into [128,1024] stores (64 gens, not 128).
  * gpsimd (SWDGE) DMAs attach trigger work to the SP sequencer ahead
    of its HWDGE queue -- never use them here.
  * The ACT sequencer must not carry early DMA gens or the lazy
    ACT_TABLE_LOAD: both delay the -rxf_r casts that gate the 4th
    matmul of every GEMM tile (a warm square forces the table load
    into the preamble).
  * The PE clock (HAM gate) reaches 2.4GHz only after ~3.4us with no
    instruction gap >~0.2us; any later gap drops it back to 1.2GHz
    for ~3.4us. Warmup + filler matmuls keep the stream gapless from
    ~7.2us until the GEMM stream is self-sustaining.
  * Main loop is n-outer/m-inner so freq group g is first needed
    ~6.9us*g into the stream; the freq DMAs stream in behind the
    critical head pack on the same (sync) ring.
"""

import numpy as np
import ml_dtypes

B = 8192
T = 8192
L = 128
NCORES = 8
BPC = B // NCORES  # batch rows per core

_BF16 = ml_dtypes.bfloat16

_CACHE = {}


# --------------------------------------------------------------------------- #
# Custom DVE op: out = (Src0^2 + Src1) * C0
# (Src0 = Sr from PSUM, Src1 = Si^2 staged by ScalarE, C0 = 1/temperature)
# --------------------------------------------------------------------------- #
def _get_sqadd_op():
    import concourse.dve_ops as dve_ops
    from concourse.dve_spec import Spec, Src0, Src1, C0, sq, lower, _has_src1
    from concourse.dve_uop import DveOpSpec

    name = "SQ_ADD_SCALE_ANT"
    for op in dve_ops.OPS:
        if op.name == name:
            return op

    spec = Spec(
        body=(sq(Src0) + Src1) * C0,
        reference=lambda in0, in1, s0, s1, imm2: (
            (in0.astype(np.float32) ** 2 + in1.astype(np.float32)) * s0
        ).astype(np.float32),
    )
    opcode = dve_ops._CUSTOM_DVE_ROW_BASE + len(dve_ops.OPS)
    assert opcode < 0x20
    shas = {}
    for ver in ("v3", "v4"):
        compiled = DveOpSpec(
            name=name, opcode=opcode, uops=lower(spec, ver=ver), rd1_en=_has_src1(spec)
        )
        shas[ver] = compiled.sha(ver)
    op = dve_ops.DveOp(name, spec, subdim=False, uops_sha=shas)
    dve_ops.OPS.append(op)
    dve_ops.CUSTOM_DVE_SPECS[name] = spec
    dve_ops._SUB_OPCODE_FOR_NAME[name] = opcode
    return op


# Packed head1: [W_r | W_ni | W_i | rx_r c0 | rx_i c0]     -> 3*128 + 2*512
H1_COLS = 3 * L + 2 * 512
# Packed head2: [g0_r | g0_i | temp(2 cols of raw bits) | pad]
H2_COLS = 2 * 1024 + 4


# --------------------------------------------------------------------------- #
# Bass program (one SPMD NeuronCore)
# --------------------------------------------------------------------------- #
def build_nc(bpc=BPC, t=T, debug=False):
    from contextlib import ExitStack

    import concourse.bacc as bacc
    import concourse.bass as bass
    import concourse.mybir as mybir
    import concourse.tile as tile

    f32 = mybir.dt.float32
    bf16 = mybir.dt.bfloat16
    sqadd = _get_sqadd_op()

    NG = 512   # output columns per PSUM group (1 bank)
    FG = 1024  # freq columns per group
    HC = 512   # rx/DFT column chunk
    assert bpc % HC == 0 and t % FG == 0
    n_chunks = bpc // HC
    n_groups = t // NG
    m_tiles = bpc // 128
    gtot = t // FG

    nc = bacc.Bacc("TRN2", target_bir_lowering=False, debug=debug, num_devices=NCORES)

    head1 = nc.dram_tensor("head1", [L, H1_COLS], bf16, kind="ExternalInput")
    head2 = nc.dram_tensor("head2", [L, H2_COLS], bf16, kind="ExternalInput")
    rxp1 = nc.dram_tensor("rxp1", [L, 2 * HC], bf16, kind="ExternalInput")
    fqt = nc.dram_tensor("fqt", [L, 2 * t], bf16, kind="ExternalInput")
    out = nc.dram_tensor("out", [bpc, t], bf16, kind="ExternalOutput")

    with tile.TileContext(nc) as tc, ExitStack() as ctx:
        consts = ctx.enter_context(tc.tile_pool(name="consts", bufs=1))
        psum = ctx.enter_context(
            tc.tile_pool(name="psum", bufs=4, space=bass.MemorySpace.PSUM)
        )
        sq_pool = ctx.enter_context(tc.tile_pool(name="sq", bufs=6))
        out_pool = ctx.enter_context(tc.tile_pool(name="ob", bufs=12))

        # ---- PE + ACT warmup ------------------------------------------ #
        # Raw sbuf tensor: no memset, no deps, PE busy from ~7.2us.
        warm_w_t = nc.alloc_sbuf_tensor("warm_w_raw", [128, 128], bf16)
        warm_w = warm_w_t[:, :]
        warm_ps = psum.tile([128, NG], mybir.dt.float32, tag="si")
        for _ in range(40):
            nc.tensor.matmul(warm_ps[:, 0:128], warm_w, warm_w, start=True, stop=True)
        act_scratch = sq_pool.tile([128, NG], f32, name="act_scratch", bufs=1)
        nc.scalar.square(act_scratch[:, 0:128], warm_w)

        # ---- load inputs (all on the SP/sync HWDGE ring, need-order) -- #
        h1_sb = consts.tile([L, H1_COLS], bf16)
        nc.sync.dma_start(h1_sb[:], head1[:, :])
        h2_sb = consts.tile([L, H2_COLS], bf16)
        nc.sync.dma_start(h2_sb[:], head2[:, :])
        rxp1_sb = consts.tile([L, 2 * HC], bf16)
        if n_chunks > 1:
            nc.sync.dma_start(rxp1_sb[:], rxp1[:, :])
        fq_sb = []
        for g in range(1, gtot):
            fq = consts.tile([L, 2 * FG], bf16, tag=f"fq{g}", name=f"fq{g}")
            nc.sync.dma_start(fq[:], fqt[:, g * 2 * FG : (g + 1) * 2 * FG])
            fq_sb.append(fq)

        wr = h1_sb[:, 0:L]
        wni = h1_sb[:, L : 2 * L]
        wi = h1_sb[:, 2 * L : 3 * L]

        def rx_slice(c, imag):
            if c == 0:
                base = 3 * L + imag * HC
                return h1_sb[:, base : base + HC]
            return rxp1_sb[:, imag * HC : (imag + 1) * HC]

        def f_slice(g, j, imag):
            if g == 0:
                base = imag * FG + j * NG
                return h2_sb[:, base : base + NG]
            return fq_sb[g - 1][:, imag * FG + j * NG : imag * FG + (j + 1) * NG]

        # ---- DFT of rx (bf16): rxfT = W @ rxT ------------------------- #
        # W symmetric, so PE's lhsT is W itself.
        # rxfT_r = Wr@rxT_r - Wi@rxT_i ; rxfT_i = Wr@rxT_i + Wi@rxT_r
        rxf_r = consts.tile([L, bpc], bf16)
        rxf_i = consts.tile([L, bpc], bf16)
        rxf_nr = consts.tile([L, bpc], bf16)  # -rxfT_r

        def emit_dft(c):
            rr = rx_slice(c, 0)
            ri = rx_slice(c, 1)
            pr = psum.tile([128, NG], mybir.dt.float32, tag="sr")
            nc.tensor.matmul(pr[:, 0:HC], wr, rr, start=True, stop=False)
            nc.tensor.matmul(pr[:, 0:HC], wni, ri, start=False, stop=True)
            pi = psum.tile([128, NG], mybir.dt.float32, tag="si")
            nc.tensor.matmul(pi[:, 0:HC], wr, ri, start=True, stop=False)
            nc.tensor.matmul(pi[:, 0:HC], wi, rr, start=False, stop=True)
            return pr, pi

        def emit_casts(c, pr, pi, step):
            # DVE: +rxf_r, rxf_i ; ACT: -rxf_r. Chunked so the first main
            # matmuls only gate on their own 128-col slice.
            for k0 in range(0, HC, step):
                ks = slice(c * HC + k0, c * HC + k0 + step)
                kp = slice(k0, k0 + step)
                nc.vector.tensor_copy(rxf_r[:, ks], pr[:, kp])
                nc.vector.tensor_copy(rxf_i[:, ks], pi[:, kp])
                nc.scalar.mul(rxf_nr[:, ks], pr[:, kp], -1.0)

        pr0, pi0 = emit_dft(0)
        emit_casts(0, pr0, pi0, 256)
        # filler warmups bridge the DFT->GEMM handoff so the PE stream
        # stays gapless while the casts and g0 land
        for _ in range(13):
            nc.tensor.matmul(warm_ps[:, 0:128], warm_w, warm_w, start=True, stop=True)

        invt_sb = consts.tile([128, 1], f32)
        nc.vector.reciprocal(invt_sb[:], h2_sb[:, 2 * FG : 2 * FG + 2].bitcast(f32))

        # ---- main complex GEMM + fused |.|^2 epilogue ----------------- #
        # Sr = rxf_r.T @ fr + rxf_i.T @ fi
        # Si = rxf_i.T @ fr - rxf_r.T @ fi
        ndma = 0
        ob_store = {}
        for n in range(n_groups):
            g, j = divmod(n, FG // NG)
            for m in range(m_tiles):
                if n == 0 and m == 2 and n_chunks > 1:
                    # second DFT chunk slots in while chunk-0 rows compute
                    pr1, pi1 = emit_dft(1)
                    emit_casts(1, pr1, pi1, 256)
                ms = slice(m * 128, (m + 1) * 128)
                # si pair first: the ACT square then overlaps the sr matmuls
                # instead of serializing after them
                si = psum.tile([128, NG], mybir.dt.float32, tag="si")
                sr = psum.tile([128, NG], mybir.dt.float32, tag="sr")
                fr_ap = f_slice(g, j, 0)
                fi_ap = f_slice(g, j, 1)
                nc.tensor.matmul(si[:], rxf_i[:, ms], fr_ap, start=True, stop=False)
                nc.tensor.matmul(si[:], rxf_nr[:, ms], fi_ap, start=False, stop=True)
                nc.tensor.matmul(sr[:], rxf_r[:, ms], fr_ap, start=True, stop=False)
                nc.tensor.matmul(sr[:], rxf_i[:, ms], fi_ap, start=False, stop=True)
                t2 = sq_pool.tile([128, NG], f32)
                nc.scalar.square(t2[:], si[:])
                if n >= n_groups - 2:
                    # tail groups: unpaired stores so the kernel-exit drain
                    # only waits on a single [128,512] transfer
                    obs = out_pool.tile(
                        [128, NG], bf16, tag="ob2", name=f"obl{n}_{m}", bufs=6
                    )
                    nc.vector._custom_dve(
                        sqadd, out=obs[:], in0=sr[:], in1=t2[:], s0=invt_sb[:]
                    )
                    oeng = nc.sync if ndma % 2 == 0 else nc.scalar
                    ndma += 1
                    oeng.dma_start(out[ms, n * NG : (n + 1) * NG], obs[:])
                else:
                    half = n % 2
                    if half == 0:
                        ob_store[m] = out_pool.tile(
                            [128, 2 * NG], bf16, tag="ob", name=f"ob{n}_{m}"
                        )
                    ob = ob_store[m]
                    nc.vector._custom_dve(
                        sqadd,
                        out=ob[:, half * NG : (half + 1) * NG],
                        in0=sr[:],
                        in1=t2[:],
                        s0=invt_sb[:],
                    )
                    if half == 1:
                        oeng = nc.sync if ndma % 2 == 0 else nc.scalar
                        ndma += 1
                        oeng.dma_start(out[ms, (n - 1) * NG : (n + 1) * NG], ob[:])

    nc.compile()
    return nc


def _host_prep(rx_real, rx_imag, freq_real, freq_imag, temperature, bpc=BPC, t=T):
    """Layout marshaling only: shard/transpose/cast/pack inputs."""
    lk = np.outer(np.arange(L), np.arange(L)).astype(np.float64)
    w = np.exp(-2j * np.pi * lk / L) / np.sqrt(L)  # ortho DFT matrix (symmetric)
    w_r = w.real.astype(np.float32).astype(_BF16)
    w_i = w.imag.astype(np.float32).astype(_BF16)
    w_ni = (-w.imag.astype(np.float32)).astype(_BF16)

    fq_r = freq_real[:t].T.astype(_BF16)  # [L, T]
    fq_i = freq_imag[:t].T.astype(_BF16)
    # fqt packed per group g: [fr_g | fi_g], each FG=1024 cols
    FG = 1024
    gtot = t // FG
    fqt = np.empty((L, 2 * t), dtype=_BF16)
    for g in range(gtot):
        fqt[:, g * 2 * FG : g * 2 * FG + FG] = fq_r[:, g * FG : (g + 1) * FG]
        fqt[:, g * 2 * FG + FG : (g + 1) * 2 * FG] = fq_i[:, g * FG : (g + 1) * FG]
    fqt = np.ascontiguousarray(fqt)

    temp_bits = (
        np.full((L, 1), np.asarray(temperature), np.float32).view(_BF16)
    )  # [L, 2] raw fp32 bit pattern

    rxt_r = np.asarray(rx_real, np.float32).T.astype(_BF16)  # [L, B]
    rxt_i = np.asarray(rx_imag, np.float32).T.astype(_BF16)

    HC = 512
    head2 = np.empty((L, H2_COLS), dtype=_BF16)
    head2[:, 0:FG] = fqt[:, 0:FG]
    head2[:, FG : 2 * FG] = fqt[:, FG : 2 * FG]
    head2[:, 2 * FG : 2 * FG + 2] = temp_bits
    head2[:, 2 * FG + 2 :] = 0
    head2 = np.ascontiguousarray(head2)

    in_maps = []
    for c in range(NCORES):
        cs = slice(c * bpc, (c + 1) * bpc)
        rr = rxt_r[:, cs]
        ri = rxt_i[:, cs]
        head1 = np.empty((L, H1_COLS), dtype=_BF16)
        head1[:, 0:L] = w_r
        head1[:, L : 2 * L] = w_ni
        head1[:, 2 * L : 3 * L] = w_i
        head1[:, 3 * L : 3 * L + HC] = rr[:, 0:HC]
        head1[:, 3 * L + HC : 3 * L + 2 * HC] = ri[:, 0:HC]
        rxp1 = np.empty((L, 2 * HC), dtype=_BF16)
        rxp1[:, 0:HC] = rr[:, HC : 2 * HC]
        rxp1[:, HC : 2 * HC] = ri[:, HC : 2 * HC]
        in_maps.append(
            {
                "head1": np.ascontiguousarray(head1),
                "head2": head2,
                "rxp1": np.ascontiguousarray(rxp1),
                "fqt": fqt,
            }
        )
    return in_maps


def kernel(rx_real, rx_imag, freq_real, freq_imag, temperature):
    from concourse.bass_utils import run_bass_kernel_spmd

    if "nc" not in _CACHE:
        _CACHE["nc"] = build_nc()
    nc = _CACHE["nc"]

    in_maps = _host_prep(rx_real, rx_imag, freq_real, freq_imag, temperature)
    res = run_bass_kernel_spmd(nc, in_maps, core_ids=list(range(NCORES)))
    _CACHE["last_result"] = res
    full = np.concatenate([r["out"] for r in res.results], axis=0)
    return np.ascontiguousarray(full.astype(np.float32))
